# revision 9
# baseline (speedup 1.0000x reference)
"""Multi-head attention (B=2, L=2048, D=1024, H=16) on 8 TRN2 NeuronCores.

Sharding: batch x head-group. Core c handles batch c//4 and heads
4*(c%4) .. 4*(c%4)+3. Each core:
  - projects its q/k/v slices (transposed activations fed from host),
  - runs flash-style attention in the "S-transposed" layout
    (keys on partitions, queries on free dim) so no on-device transposes
    are ever needed,
  - computes a partial output projection against its Wo column slice.
Host sums the 4 partials per batch.

All DMA'd tensors (activations, weights, output partials) are bf16 to
halve HBM traffic; matmuls run bf16 (full PE rate at moving dim>=256)
with f32 PSUM accumulation. Softmax uses exp without max-subtraction
(scores are O(1) by construction); the attention mask folds into the
exp bias, and the softmax denominator comes for free from a ones-row
appended to V.

Emission is software-pipelined: group 0's projections run at 256-token
granularity ordered to match DMA arrival (wq, xq, wk, xk, wv, xv), so
the PE starts ~2.5us in; later projection groups are chunked by
512-token groups with the first two attention groups streaming behind
the input DMA.
"""
import sys

sys.path.insert(0, "/opt/trn_rl_repo")

import numpy as np
import ml_dtypes
from contextlib import ExitStack

import concourse.bass as bass
import concourse.mybir as mybir
import concourse.tile as tile
from concourse import bacc
from concourse.bass import ts
from concourse.bass_utils import run_bass_kernel_spmd

F32 = mybir.dt.float32
BF16 = mybir.dt.bfloat16
I32 = mybir.dt.int32
EXP = mybir.ActivationFunctionType.Exp
BF = ml_dtypes.bfloat16

B = 2
L = 2048
D = 1024
H = 16
DH = 64
HG = 4          # heads per core
NC = 8          # cores
P = 128
DT = D // P     # 8 d-tiles
JT = L // P     # 16 key tiles
IC = L // 512   # 4 query chunks of 512
G = 4           # projection token groups (512 tokens each)

_BUILT = {}


def _build(with_loop=False):
    nc = bacc.Bacc("TRN2", target_bir_lowering=False, debug=False, num_devices=1)

    xqT_d = nc.dram_tensor("xqT", (D, L), BF16, kind="ExternalInput").ap()
    xkT_d = nc.dram_tensor("xkT", (D, L), BF16, kind="ExternalInput").ap()
    xvT_d = nc.dram_tensor("xvT", (D, L), BF16, kind="ExternalInput").ap()
    wqT_d = nc.dram_tensor("wqT", (D, HG * DH), BF16, kind="ExternalInput").ap()
    wkT_d = nc.dram_tensor("wkT", (D, HG * DH), BF16, kind="ExternalInput").ap()
    wvT_d = nc.dram_tensor("wvT", (D, HG * DH), BF16, kind="ExternalInput").ap()
    woT_d = nc.dram_tensor("woT", (HG * DH, D), BF16, kind="ExternalInput").ap()
    mb_d = nc.dram_tensor("mb", (P, JT), F32, kind="ExternalInput").ap()
    out_d = nc.dram_tensor("partial", (L, D), BF16, kind="ExternalOutput").ap()
    if with_loop:
        nl_d = nc.dram_tensor("nloop", (1, 1), I32, kind="ExternalInput").ap()

    xq_v = xqT_d.rearrange("(dt p) t -> p dt t", p=P)
    xk_v = xkT_d.rearrange("(dt p) t -> p dt t", p=P)
    xv_v = xvT_d.rearrange("(dt p) t -> p dt t", p=P)

    marks = []

    def mark(label):
        marks.append((label, int(nc.get_next_instruction_name().split("-")[1])))

    with tile.TileContext(nc) as tc, ExitStack() as ctx:
        perm = ctx.enter_context(tc.tile_pool(name="perm", bufs=1))

        # resident weights
        wq = perm.tile([P, DT, HG * DH], BF16)
        wk = perm.tile([P, DT, HG * DH], BF16)
        wv = perm.tile([P, DT, HG * DH], BF16)
        wo = perm.tile([P, 2, D], BF16)
        mb = perm.tile([P, JT], F32)

        QT = [perm.tile([P, 2, 512], BF16, tag=f"QT{g}", name=f"QT{g}") for g in range(G)]
        KT = [perm.tile([P, 2, 512], BF16, tag=f"KT{g}", name=f"KT{g}") for g in range(G)]
        VT = [perm.tile([P, HG * (DH + 1)], BF16, tag=f"VT{j}", name=f"VT{j}") for j in range(JT)]
        OT = [perm.tile([P, 2, 512], BF16, tag=f"OT{g}", name=f"OT{g}") for g in range(G)]

        # ones columns of V (softmax denominator rows) — written once,
        # never clobbered by the V projection copies.
        for j in range(JT):
            vg = VT[j].rearrange("p (h c) -> p h c", c=DH + 1)
            nc.gpsimd.memset(vg[:, :, DH:DH + 1], 1.0)

        if with_loop:
            nl_sb = perm.tile([1, 1], I32)
            nc.sync.dma_start(nl_sb[:], nl_d[:])
            handles = []
            for eng in (nc.gpsimd, nc.scalar, nc.tensor, nc.vector, nc.sync):
                r = eng.alloc_register(f"nl_{eng.engine.name}")
                eng.reg_load(r, nl_sb[0:1, 0:1])
                handles.append(r)
            n_val = nc.snap(bass.RegisterHandles(handles), min_val=1, max_val=1 << 20)
            loop_cm = tc.For_i(0, n_val)
        else:
            loop_cm = None

        xpool = ctx.enter_context(tc.tile_pool(name="xg", bufs=4))
        spool = ctx.enter_context(tc.tile_pool(name="spool", bufs=2, space="PSUM"))
        ptpool = ctx.enter_context(tc.tile_pool(name="pt", bufs=3))
        stpool = ctx.enter_context(tc.tile_pool(name="st", bufs=2))
        small = ctx.enter_context(tc.tile_pool(name="small", bufs=2))

        def body():
            # Scores-slot rotation: pool A (2 gens) during ramp; steady state
            # adds pool B (1 gen) for an effective 3-slot rotation that also
            # feeds the output-projection PSUM.
            rot = {"pools": [spool, spool], "i": 0}

            def sslot():
                i = rot["i"] % len(rot["pools"])
                rot["i"] += 1
                return rot["pools"][i].tile([P, 1024], F32, tag="s", name="s")

            def xg_tiles():
                xq_g = xpool.tile([P, DT, 512], BF16, tag="xg", name="xq_g")
                xk_g = xpool.tile([P, DT, 512], BF16, tag="xg", name="xk_g")
                xv_g = xpool.tile([P, DT, 512], BF16, tag="xg", name="xv_g")
                return xq_g, xk_g, xv_g

            def proj_qk_half(src, wt, dst, h):
                # 256-token half-group projection for Q/K (ramp only).
                ps = sslot()
                for p in range(2):
                    for d in range(DT):
                        nc.tensor.matmul(
                            ps[:, p * 512 + h * 256:p * 512 + h * 256 + 256],
                            wt[:, d, ts(p, P)], src[:, d, ts(h, 256)],
                            start=(d == 0), stop=(d == DT - 1),
                        )
                    nc.vector.tensor_copy(
                        dst[:, p, h * 256:h * 256 + 256],
                        ps[:, p * 512 + h * 256:p * 512 + h * 256 + 256],
                    )

            def proj_v_jt(xv_g, jt):
                psv = sslot()
                for d in range(DT):
                    nc.tensor.matmul(
                        psv[:, 0:HG * DH],
                        xv_g[:, d, ts(jt % 4, P)], wv[:, d, :],
                        start=(d == 0), stop=(d == DT - 1),
                    )
                vg = VT[jt].rearrange("p (h c) -> p h c", c=DH + 1)
                nc.vector.tensor_copy(
                    vg[:, :, 0:DH],
                    psv[:, 0:HG * DH].rearrange("p (h c) -> p h c", c=DH),
                )

            def proj_group(g, xq_g, xk_g, xv_g):
                # Q and K: out [pair-heads on partitions, 512 tokens]
                for src, wt, dst in ((xq_g, wq, QT[g]), (xk_g, wk, KT[g])):
                    ps = sslot()
                    for d in range(DT):
                        for p in range(2):
                            nc.tensor.matmul(
                                ps[:, ts(p, 512)], wt[:, d, ts(p, P)], src[:, d, :],
                                start=(d == 0), stop=(d == DT - 1),
                            )
                    for p in range(2):
                        nc.vector.tensor_copy(dst[:, p, :], ps[:, ts(p, 512)])
                for jt in range(4 * g, 4 * g + 4):
                    proj_v_jt(xv_g, jt)

            # PV matmuls lag one j behind their exp in the PE stream so the PE
            # never stalls on the current j's exp; norms and output-projection
            # blocks are sprinkled into the NEXT group's j-loop as PE filler.
            PENDING = []   # [(hp, pvs, j, pt)]
            FILLER = []    # deferred closures (norm halves / oproj blocks)

            def flush_pv(keep):
                while len(PENDING) > keep:
                    hp, grp, j, pt = PENDING.pop(0)
                    qh = grp.get("qh")
                    for h01 in range(2):
                        h = 2 * hp + h01
                        if qh is None:
                            rhs = pt[:, ts(h01, 512)]
                        else:
                            q0 = h01 * 512 + qh * 256
                            rhs = pt[:, q0:q0 + 256]
                        nc.tensor.matmul(
                            grp["pvs"][h01][:],
                            VT[j][:, h * (DH + 1):(h + 1) * (DH + 1)],
                            rhs,
                            start=(j == 0), stop=(j == JT - 1),
                        )

            def attn_jseg(hp, ic, grp, j_range, fill=True):
                # grp["qh"]: None = full 512-query chunk; 0/1 = 256-query half
                qh = grp.get("qh")
                for j in j_range:
                    ps = sslot()
                    if qh is None:
                        nc.tensor.matmul(
                            ps[:, 0:512],
                            KT[j // 4][0:DH, hp, ts(j % 4, P)],
                            QT[ic][0:DH, hp, :],
                            start=True, stop=True,
                        )
                        nc.tensor.matmul(
                            ps[:, 512:1024],
                            KT[j // 4][DH:P, hp, ts(j % 4, P)],
                            QT[ic][DH:P, hp, :],
                            start=True, stop=True, tile_position=(DH, 0),
                        )
                        pin, pout_sl = ps[:], (slice(0, 1024),)
                    else:
                        q0 = qh * 256
                        nc.tensor.matmul(
                            ps[:, q0:q0 + 256],
                            KT[j // 4][0:DH, hp, ts(j % 4, P)],
                            QT[ic][0:DH, hp, q0:q0 + 256],
                            start=True, stop=True,
                        )
                        nc.tensor.matmul(
                            ps[:, 512 + q0:512 + q0 + 256],
                            KT[j // 4][DH:P, hp, ts(j % 4, P)],
                            QT[ic][DH:P, hp, q0:q0 + 256],
                            start=True, stop=True, tile_position=(DH, 0),
                        )
                        pin = ps.rearrange("p (h q) -> p h q", h=2)[:, :, q0:q0 + 256]
                    pt = ptpool.tile([P, 1024], BF16, tag="pt", name="pt")
                    if qh is None:
                        nc.scalar.activation(
                            pt[:], pin, EXP, bias=mb[:, j:j + 1], scale=0.125,
                        )
                    else:
                        q0 = qh * 256
                        pto = pt.rearrange("p (h q) -> p h q", h=2)[:, :, q0:q0 + 256]
                        nc.scalar.activation(
                            pto, pin, EXP, bias=mb[:, j:j + 1], scale=0.125,
                        )
                    PENDING.append((hp, grp, j, pt))
                    flush_pv(1)
                    if fill and FILLER:
                        FILLER.pop(0)()

            def attn_norm(hp, ic, grp):
                qh = grp.get("qh")
                q0, qn = (0, 512) if qh is None else (qh * 256, 256)
                for h01 in range(2):
                    pv = grp["pvs"][h01]
                    rec = small.tile([1, 512], F32, tag="rec", name="rec")
                    nc.vector.reciprocal(rec[:, 0:qn], pv[DH:DH + 1, :])
                    bc = small.tile([DH, 512], F32, tag="bc", name="bc")
                    nc.gpsimd.partition_broadcast(bc[:, 0:qn], rec[:, 0:qn])
                    nc.vector.tensor_mul(
                        out=OT[ic][ts(h01, DH), hp, q0:q0 + qn],
                        in0=pv[0:DH, :], in1=bc[:, 0:qn],
                    )

            def new_grp(pvpool, qh=None):
                qn = 512 if qh is None else 256
                return {
                    "qh": qh,
                    "pvs": [
                        pvpool.tile([DH + 1, qn], F32, tag="pvA", name="pvA"),
                        pvpool.tile([DH + 1, qn], F32, tag="pvB", name="pvB"),
                    ],
                }

            def oproj(tb):
                ic = tb // 4
                st = stpool.tile([P, D], BF16, tag="st", name="st")
                pso = rot["oproj"].tile([P, 1024], F32, tag="pso", name="pso")
                for mc in range(2):
                    for kt in range(2):
                        nc.tensor.matmul(
                            pso[:, ts(mc, 512)], OT[ic][:, kt, ts(tb % 4, P)],
                            wo[:, kt, ts(mc, 512)],
                            start=(kt == 0), stop=(kt == 1),
                        )
                    nc.vector.tensor_copy(st[:, ts(mc, 512)], pso[:, ts(mc, 512)])
                nc.sync.dma_start(out_d[ts(tb, P), :], st[:])

            # ---------- pipelined emission ----------
            # Ramp: DMA + projections stream behind the input DMA; group 0 at
            # 256-token granularity ordered to match DMA arrival. Attention
            # groups (0,0) and (0,1) trail. Dedicated 4-bank pv pool.
            with tc.tile_pool(name="rampv", bufs=2, space="PSUM") as rampv:
                g00 = new_grp(rampv)
                g01 = None

                xq_g, xk_g, xv_g = xg_tiles()
                nc.sync.dma_start(wq[:], wqT_d.rearrange("(dt p) m -> p dt m", p=P))
                nc.sync.dma_start(mb[:], mb_d[:])
                nc.sync.dma_start(xq_g[:, :, 0:256], xq_v[:, :, 0:256])
                nc.sync.dma_start(xq_g[:, :, 256:512], xq_v[:, :, 256:512])
                nc.sync.dma_start(wk[:], wkT_d.rearrange("(dt p) m -> p dt m", p=P))
                nc.sync.dma_start(xk_g[:, :, 0:256], xk_v[:, :, 0:256])
                nc.sync.dma_start(wv[:], wvT_d.rearrange("(dt p) m -> p dt m", p=P))
                nc.sync.dma_start(xv_g[:, :, 0:256], xv_v[:, :, 0:256])
                nc.sync.dma_start(xk_g[:, :, 256:512], xk_v[:, :, 256:512])
                nc.sync.dma_start(xv_g[:, :, 256:512], xv_v[:, :, 256:512])
                mark("dma0")

                proj_qk_half(xq_g, wq, QT[0], 0)
                proj_qk_half(xq_g, wq, QT[0], 1)
                proj_qk_half(xk_g, wk, KT[0], 0)
                proj_v_jt(xv_g, 0)
                proj_v_jt(xv_g, 1)
                attn_jseg(0, 0, g00, range(0, 2))
                proj_qk_half(xk_g, wk, KT[0], 1)
                proj_v_jt(xv_g, 2)
                proj_v_jt(xv_g, 3)
                attn_jseg(0, 0, g00, range(2, 4))
                mark("proj0")

                for g in range(1, G):
                    xq_g, xk_g, xv_g = xg_tiles()
                    nc.sync.dma_start(xq_g[:], xq_v[:, :, ts(g, 512)])
                    nc.sync.dma_start(xk_g[:], xk_v[:, :, ts(g, 512)])
                    nc.sync.dma_start(xv_g[:], xv_v[:, :, ts(g, 512)])
                    mark(f"dma{g}")
                    proj_group(g, xq_g, xk_g, xv_g)
                    mark(f"proj{g}")
                    attn_jseg(0, 0, g00, range(4 * g, 4 * g + 4))
                    if g01 is None:
                        g01 = new_grp(rampv)
                    attn_jseg(0, 1, g01, range(4 * (g - 1), 4 * g))
                    mark(f"attn_pipe{g}")
                attn_jseg(0, 1, g01, range(12, 16))
                flush_pv(0)
                attn_norm(0, 0, g00)
                attn_norm(0, 1, g01)
                mark("ramp_end")

            nc.sync.dma_start(wo[:], woT_d.rearrange("(kt p) m -> p kt m", p=P))

            # Steady state: 3-way scores-slot rotation (pool B opens in the
            # banks the ramp pv pool vacated); norms and output projections
            # fill the next group's j-loop. The final group runs as two
            # 256-query halves so its norm + output projection overlap the
            # second half instead of draining serially.
            with tc.tile_pool(name="spoolB", bufs=1, space="PSUM") as spoolB, \
                 tc.tile_pool(name="stpv", bufs=1, space="PSUM") as stpv:
                rot["oproj"] = spoolB

                def finish_group(hp, ic, grp, tbs):
                    def _norm():
                        # flush this group's remaining PVs (FIFO head) but not
                        # the already-pending PVs of the group that follows it
                        while any(g is grp for _, g, _, _ in PENDING):
                            flush_pv(len(PENDING) - 1)
                        attn_norm(hp, ic, grp)
                        for tb in tbs:
                            FILLER.append(lambda tb=tb: oproj(tb))
                    FILLER.append(_norm)

                sched = [
                    (0, 2, None, []), (0, 3, None, []),
                    (1, 0, None, [0, 1, 2, 3]), (1, 1, None, [4, 5, 6, 7]),
                    (1, 2, None, [8, 9, 10, 11]),
                    (1, 3, 0, [12, 13]), (1, 3, 1, [14, 15]),
                ]
                prev = None
                for hp, ic, qh, tbs in sched:
                    grp = new_grp(stpv, qh)
                    if prev is not None:
                        attn_jseg(prev[0], prev[1], prev[2], range(JT))
                        finish_group(*prev)
                        mark(f"attn{prev[0]}{prev[1]}{prev[3] and 'o' or ''}")
                    prev = (hp, ic, grp, tbs)
                attn_jseg(prev[0], prev[1], prev[2], range(JT))
                finish_group(*prev)
                mark("attn_last")
                flush_pv(0)
                while FILLER:
                    FILLER.pop(0)()
                mark("drain")

        if loop_cm is not None:
            with loop_cm:
                body()
        else:
            body()

    nc.compile()
    nc._phase_marks = marks
    return nc


def _in_maps(q, k, v, attention_mask, Wq, Wk, Wv, Wo, nloop=None):
    q = np.asarray(q, dtype=np.float32)
    k = np.asarray(k, dtype=np.float32)
    v = np.asarray(v, dtype=np.float32)
    Wq = np.asarray(Wq, dtype=np.float32)
    Wk = np.asarray(Wk, dtype=np.float32)
    Wv = np.asarray(Wv, dtype=np.float32)
    Wo = np.asarray(Wo, dtype=np.float32)
    mask = np.asarray(attention_mask)

    xT = {}
    for b in range(B):
        xT[("q", b)] = np.ascontiguousarray(q[b].T).astype(BF)
        xT[("k", b)] = np.ascontiguousarray(k[b].T).astype(BF)
        xT[("v", b)] = np.ascontiguousarray(v[b].T).astype(BF)

    in_maps = []
    for c in range(NC):
        b, hg = c // HG, c % HG
        rows = slice(hg * HG * DH, (hg + 1) * HG * DH)
        mbn = np.where(mask[b] == 0, np.float32(-1e9), np.float32(0.0))
        m = {
            "xqT": xT[("q", b)],
            "xkT": xT[("k", b)],
            "xvT": xT[("v", b)],
            "wqT": np.ascontiguousarray(Wq[rows].T).astype(BF),
            "wkT": np.ascontiguousarray(Wk[rows].T).astype(BF),
            "wvT": np.ascontiguousarray(Wv[rows].T).astype(BF),
            "woT": np.ascontiguousarray(Wo[:, rows].T).astype(BF),
            "mb": np.ascontiguousarray(mbn.reshape(JT, P).T),
        }
        if nloop is not None:
            m["nloop"] = np.array([[nloop]], dtype=np.int32)
        in_maps.append(m)
    return in_maps


def kernel(q, k, v, attention_mask, Wq, Wk, Wv, Wo):
    if "plain" not in _BUILT:
        _BUILT["plain"] = _build()
    nc = _BUILT["plain"]

    in_maps = _in_maps(q, k, v, attention_mask, Wq, Wk, Wv, Wo)
    res = run_bass_kernel_spmd(nc, in_maps, core_ids=list(range(NC)))
    kernel.last_results = res

    out = np.zeros((B, L, D), dtype=np.float64)
    for c in range(NC):
        out[c // HG] += res.results[c]["partial"].astype(np.float64)
    return out.astype(np.float32)


# revision 34
# speedup vs baseline: 1.3142x; 1.3142x over previous
"""Multi-head attention (B=2, L=2048, D=1024, H=16) on 8 TRN2 NeuronCores.

Sharding: batch x head-group. Core c handles batch c//4 and heads
4*(c%4) .. 4*(c%4)+3. Each core:
  - projects its q/k/v slices (transposed activations fed from host),
  - runs flash-style attention in the "S-transposed" layout
    (keys on partitions, queries on free dim) so no on-device transposes
    are ever needed,
  - computes a partial output projection against its Wo column slice.
Host sums the 4 partials per batch.

All DMA'd tensors (activations, weights, output partials) are bf16 to
halve HBM traffic; matmuls run bf16 (full PE rate at moving dim>=256)
with f32 PSUM accumulation. Softmax uses exp without max-subtraction
(scores are O(1) by construction); the attention mask folds into the
exp bias, and the softmax denominator comes for free from a ones-row
appended to V.

Emission is software-pipelined: group 0's projections run at 256-token
granularity ordered to match DMA arrival (wq, xq, wk, xk, wv, xv), so
the PE starts ~2.5us in; later projection groups are chunked by
512-token groups with the first two attention groups streaming behind
the input DMA.
"""
import sys

sys.path.insert(0, "/opt/trn_rl_repo")

import numpy as np
import ml_dtypes
from contextlib import ExitStack

import concourse.bass as bass
import concourse.mybir as mybir
import concourse.tile as tile
from concourse import bacc
from concourse.bass import ts
from concourse.bass_utils import run_bass_kernel_spmd

F32 = mybir.dt.float32
BF16 = mybir.dt.bfloat16
I32 = mybir.dt.int32
EXP = mybir.ActivationFunctionType.Exp
BF = ml_dtypes.bfloat16

B = 2
L = 2048
D = 1024
H = 16
DH = 64
HG = 4          # heads per core
NC = 8          # cores
P = 128
DT = D // P     # 8 d-tiles
JT = L // P     # 16 key tiles
IC = L // 512   # 4 query chunks of 512
G = 4           # projection token groups (512 tokens each)

_BUILT = {}


def _build(with_loop=False):
    nc = bacc.Bacc("TRN2", target_bir_lowering=False, debug=False, num_devices=1)

    # Host pre-tiles every input into the exact SBUF layout so each DMA row
    # is a large contiguous run (few fat descriptors instead of thousands of
    # 512B ones): x tensors as [group*P, DT*512], weights as [P, DT*M].
    xqT_d = nc.dram_tensor("xqT", (G * P, DT * 512), BF16, kind="ExternalInput").ap()
    xkT_d = nc.dram_tensor("xkT", (G * P, DT * 512), BF16, kind="ExternalInput").ap()
    xvT_d = nc.dram_tensor("xvT", (G * P, DT * 512), BF16, kind="ExternalInput").ap()
    wqT_d = nc.dram_tensor("wqT", (P, DT * HG * DH), BF16, kind="ExternalInput").ap()
    wkT_d = nc.dram_tensor("wkT", (P, DT * HG * DH), BF16, kind="ExternalInput").ap()
    wvT_d = nc.dram_tensor("wvT", (P, DT * HG * DH), BF16, kind="ExternalInput").ap()
    woT_d = nc.dram_tensor("woT", (P, 2 * D), BF16, kind="ExternalInput").ap()
    mb_d = nc.dram_tensor("mb", (P, JT), F32, kind="ExternalInput").ap()
    # partial output packed as [qchunk*P, 4tb*D] so each DMA row is 8KB
    # contiguous (128 fat descriptors per chunk instead of 512 thin ones)
    out_d = nc.dram_tensor("partial", (IC * P, 4 * D), BF16, kind="ExternalOutput").ap()
    if with_loop:
        nl_d = nc.dram_tensor("nloop", (1, 1), I32, kind="ExternalInput").ap()

    def xq_v(g):
        return xqT_d[ts(g, P), :].rearrange("p (dt t) -> p dt t", t=512)

    def xk_v(g):
        return xkT_d[ts(g, P), :].rearrange("p (dt t) -> p dt t", t=512)

    def xv_v(g):
        return xvT_d[ts(g, P), :].rearrange("p (dt t) -> p dt t", t=512)

    wq_v = wqT_d.rearrange("p (dt m) -> p dt m", m=HG * DH)
    wk_v = wkT_d.rearrange("p (dt m) -> p dt m", m=HG * DH)
    wv_v = wvT_d.rearrange("p (dt m) -> p dt m", m=HG * DH)
    wo_v = woT_d.rearrange("p (kt m) -> p kt m", m=D)

    marks = []

    def mark(label):
        marks.append((label, int(nc.get_next_instruction_name().split("-")[1])))

    with tile.TileContext(nc) as tc, ExitStack() as ctx:
        perm = ctx.enter_context(tc.tile_pool(name="perm", bufs=1))

        # resident weights
        wq = perm.tile([P, DT, HG * DH], BF16)
        wk = perm.tile([P, DT, HG * DH], BF16)
        wv = perm.tile([P, DT, HG * DH], BF16)
        wo = perm.tile([P, 2, D], BF16)
        mb = perm.tile([P, JT], F32)

        QT = [perm.tile([P, 2, 512], BF16, tag=f"QT{g}", name=f"QT{g}") for g in range(G)]
        KT = [perm.tile([P, 2, 512], BF16, tag=f"KT{g}", name=f"KT{g}") for g in range(G)]
        VT = [perm.tile([P, HG * (DH + 1)], BF16, tag=f"VT{j}", name=f"VT{j}") for j in range(JT)]
        OT = [perm.tile([P, 2, 512], BF16, tag=f"OT{g}", name=f"OT{g}") for g in range(G)]

        # ones columns of V (softmax denominator rows) — written once,
        # never clobbered by the V projection copies.
        for j in range(JT):
            vg = VT[j].rearrange("p (h c) -> p h c", c=DH + 1)
            nc.gpsimd.memset(vg[:, :, DH:DH + 1], 1.0)

        if with_loop:
            nl_sb = perm.tile([1, 1], I32)
            nc.sync.dma_start(nl_sb[:], nl_d[:])
            handles = []
            for eng in (nc.gpsimd, nc.scalar, nc.tensor, nc.vector, nc.sync):
                r = eng.alloc_register(f"nl_{eng.engine.name}")
                eng.reg_load(r, nl_sb[0:1, 0:1])
                handles.append(r)
            n_val = nc.snap(bass.RegisterHandles(handles), min_val=1, max_val=1 << 20)
            loop_cm = tc.For_i(0, n_val)
        else:
            loop_cm = None

        xpool = ctx.enter_context(tc.tile_pool(name="xg", bufs=4))
        spool = ctx.enter_context(tc.tile_pool(name="spool", bufs=2, space="PSUM"))
        spoolB = ctx.enter_context(tc.tile_pool(name="spoolB", bufs=1, space="PSUM"))
        stpv = ctx.enter_context(tc.tile_pool(name="stpv", bufs=1, space="PSUM"))
        opool = ctx.enter_context(tc.tile_pool(name="op", bufs=1, space="PSUM"))
        ptpool = ctx.enter_context(tc.tile_pool(name="pt", bufs=44))
        stpool = ctx.enter_context(tc.tile_pool(name="st", bufs=2))
        small = ctx.enter_context(tc.tile_pool(name="small", bufs=2))

        def body():
            # Scores-slot rotation: pool A (2 gens) during ramp; steady state
            # 3-slot scores rotation: spool's two gens + spoolB.
            rot = {"i": 0}

            def sslot():
                i = rot["i"] % 3
                rot["i"] += 1
                pool = spool if i < 2 else spoolB
                return pool.tile([P, 1024], F32, tag="s", name="s")

            def xg_tiles():
                xq_g = xpool.tile([P, DT, 512], BF16, tag="xg", name="xq_g")
                xk_g = xpool.tile([P, DT, 512], BF16, tag="xg", name="xk_g")
                xv_g = xpool.tile([P, DT, 512], BF16, tag="xg", name="xv_g")
                return xq_g, xk_g, xv_g

            def proj_qk_half(src, wt, dst, h):
                # 256-token half-group projection for Q/K (ramp only).
                ps = sslot()
                for p in range(2):
                    for d in range(DT):
                        nc.tensor.matmul(
                            ps[:, p * 512 + h * 256:p * 512 + h * 256 + 256],
                            wt[:, d, ts(p, P)], src[:, d, ts(h, 256)],
                            start=(d == 0), stop=(d == DT - 1),
                        )
                    nc.vector.tensor_copy(
                        dst[:, p, h * 256:h * 256 + 256],
                        ps[:, p * 512 + h * 256:p * 512 + h * 256 + 256],
                    )

            def proj_v_jt(xv_g, jt):
                psv = sslot()
                for d in range(DT):
                    nc.tensor.matmul(
                        psv[:, 0:HG * DH],
                        xv_g[:, d, ts(jt % 4, P)], wv[:, d, :],
                        start=(d == 0), stop=(d == DT - 1),
                    )
                vg = VT[jt].rearrange("p (h c) -> p h c", c=DH + 1)
                nc.vector.tensor_copy(
                    vg[:, :, 0:DH],
                    psv[:, 0:HG * DH].rearrange("p (h c) -> p h c", c=DH),
                )
                SCHED["vt_ready"] = jt

            def proj_qk(src, wt, dst):
                # out [pair-heads on partitions, 512 tokens]
                ps = sslot()
                for d in range(DT):
                    for p in range(2):
                        nc.tensor.matmul(
                            ps[:, ts(p, 512)], wt[:, d, ts(p, P)], src[:, d, :],
                            start=(d == 0), stop=(d == DT - 1),
                        )
                for p in range(2):
                    nc.vector.tensor_copy(dst[:, p, :], ps[:, ts(p, 512)])

            # ---- decoupled attention streams (pt ring) ----
            # The exp stream (scores -> exp -> pt ring) runs freely at ACT
            # rate; the PV stream consumes pts strictly in order at 2 matmuls
            # per exp: head A of a pair in the 2nd half of its own exp window,
            # head B in the 1st half of the next pair's window. This keeps a
            # single 1-bank pv accumulator (stpv) busy with no dead time.
            def mk_pair(hp, ic, qh, unlock):
                return {"hp": hp, "ic": ic, "qh": qh, "unlock": unlock,
                        "qn": 512 if qh is None else 256,
                        "pts": {}, "exp_done": -1, "pv": [None, None]}

            P00 = mk_pair(0, 0, None, [])
            P01 = mk_pair(0, 1, None, [])
            steady = [
                mk_pair(0, 2, None, []), mk_pair(0, 3, None, []),
                mk_pair(1, 0, None, [0, 1, 2, 3]),
                mk_pair(1, 1, None, [4, 5, 6, 7]),
                mk_pair(1, 2, None, [8, 9, 10, 11]),
                mk_pair(1, 3, 0, [12, 13]), mk_pair(1, 3, 1, [14, 15]),
            ]
            PVQ = []
            for Pr in [P00, P01] + steady:
                for h01 in range(2):
                    for j in range(JT):
                        PVQ.append((Pr, h01, j))
            FILLER = []

            ST4 = {}  # ic -> 4-block staging tile

            def oproj_half(tb, mc, sttile):
                ic = tb // 4
                if ic not in ST4:
                    ST4[ic] = stpool.tile([P, 4, D], BF16, tag="st", name="st")
                st = ST4[ic]
                pso = opool.tile([P, 512], F32, tag="pso", name="pso")
                for kt in range(2):
                    nc.tensor.matmul(
                        pso[:], OT[ic][:, kt, ts(tb % 4, P)],
                        wo[:, kt, ts(mc, 512)],
                        start=(kt == 0), stop=(kt == 1),
                    )
                nc.vector.tensor_copy(st[:, tb % 4, ts(mc, 512)], pso[:])
                if tb % 4 == 3 and mc == 1:
                    nc.sync.dma_start(out_d[ts(ic, P), :],
                                      st.rearrange("p a b -> p (a b)"))

            def norm(Pr, h01):
                hp, ic, qh, qn = Pr["hp"], Pr["ic"], Pr["qh"], Pr["qn"]
                q0 = 0 if qh is None else qh * 256
                # One fast copy frees the pv PSUM bank for the next block; the
                # 3-op normalization chain runs off-critical from the copy.
                pvc = small.tile([DH + 1, 512], F32, tag="pvc", name="pvc")
                nc.vector.tensor_copy(pvc[:, 0:qn], Pr["pv"][h01][:])
                rec = small.tile([1, 512], F32, tag="rec", name="rec")
                nc.vector.reciprocal(rec[:, 0:qn], pvc[DH:DH + 1, 0:qn])
                bc = small.tile([DH, 512], F32, tag="bc", name="bc")
                nc.gpsimd.partition_broadcast(bc[:, 0:qn], rec[:, 0:qn])
                nc.vector.tensor_mul(
                    out=OT[ic][ts(h01, DH), hp, q0:q0 + qn],
                    in0=pvc[0:DH, 0:qn], in1=bc[:, 0:qn],
                )
                if h01 == 1:
                    for tb in Pr["unlock"]:
                        sttile = {}
                        for mc in range(2):
                            FILLER.append(
                                lambda tb=tb, mc=mc, st=sttile: oproj_half(tb, mc, st))

            SCHED = {"vt_ready": -1}

            def pv_pump(cur_P=None, cur_j=None, budget=2):
                done = 0
                while PVQ and (budget is None or done < budget):
                    Pr, h01, j = PVQ[0]
                    if j > Pr["exp_done"] or j > SCHED["vt_ready"]:
                        break
                    if cur_P is Pr and j == cur_j:
                        break   # stay one exp behind the ACT stream
                    PVQ.pop(0)
                    if Pr["pv"][h01] is None:
                        Pr["pv"][h01] = stpv.tile(
                            [DH + 1, Pr["qn"]], F32, tag="pv", name="pv")
                    pt = Pr["pts"][j]
                    h = 2 * Pr["hp"] + h01
                    if Pr["qh"] is None:
                        rhs = pt[:, ts(h01, 512)]
                    else:
                        q0 = h01 * 512 + Pr["qh"] * 256
                        rhs = pt[:, q0:q0 + 256]
                    nc.tensor.matmul(
                        Pr["pv"][h01][:],
                        VT[j][:, h * (DH + 1):(h + 1) * (DH + 1)], rhs,
                        start=(j == 0), stop=(j == JT - 1),
                    )
                    if j == JT - 1:
                        norm(Pr, h01)
                    done += 1

            def attn_jseg(Pr, j_range, budget=2):
                hp, ic, qh = Pr["hp"], Pr["ic"], Pr["qh"]
                for j in j_range:
                    ps = sslot()
                    if qh is None:
                        nc.tensor.matmul(
                            ps[:, 0:512],
                            KT[j // 4][0:DH, hp, ts(j % 4, P)],
                            QT[ic][0:DH, hp, :],
                            start=True, stop=True,
                        )
                        nc.tensor.matmul(
                            ps[:, 512:1024],
                            KT[j // 4][DH:P, hp, ts(j % 4, P)],
                            QT[ic][DH:P, hp, :],
                            start=True, stop=True, tile_position=(DH, 0),
                        )
                        pin = ps[:]
                    else:
                        q0 = qh * 256
                        nc.tensor.matmul(
                            ps[:, q0:q0 + 256],
                            KT[j // 4][0:DH, hp, ts(j % 4, P)],
                            QT[ic][0:DH, hp, q0:q0 + 256],
                            start=True, stop=True,
                        )
                        nc.tensor.matmul(
                            ps[:, 512 + q0:512 + q0 + 256],
                            KT[j // 4][DH:P, hp, ts(j % 4, P)],
                            QT[ic][DH:P, hp, q0:q0 + 256],
                            start=True, stop=True, tile_position=(DH, 0),
                        )
                        pin = ps.rearrange("p (h q) -> p h q", h=2)[:, :, q0:q0 + 256]
                    pt = ptpool.tile([P, 1024], BF16, tag="pt", name="pt")
                    if qh is None:
                        nc.scalar.activation(
                            pt[:], pin, EXP, bias=mb[:, j:j + 1], scale=0.125,
                        )
                    else:
                        q0 = qh * 256
                        pto = pt.rearrange("p (h q) -> p h q", h=2)[:, :, q0:q0 + 256]
                        nc.scalar.activation(
                            pto, pin, EXP, bias=mb[:, j:j + 1], scale=0.125,
                        )
                    Pr["pts"][j] = pt
                    Pr["exp_done"] = j
                    pv_pump(Pr, j, budget)
                    if FILLER:
                        FILLER.pop(0)()

            # ---------- pipelined emission ----------
            # Ramp: DMA + projections stream behind the input DMA; group 0 at
            # 256-token granularity ordered to match DMA arrival; attention
            # pairs (0,0) and (0,1) trail the projections.
            xq_g, xk_g, xv_g = xg_tiles()
            nc.sync.dma_start(wq[:], wq_v)
            nc.sync.dma_start(mb[:], mb_d[:])
            nc.sync.dma_start(xq_g[:, :, 0:256], xq_v(0)[:, :, 0:256])
            nc.sync.dma_start(xq_g[:, :, 256:512], xq_v(0)[:, :, 256:512])
            nc.sync.dma_start(wk[:], wk_v)
            nc.sync.dma_start(xk_g[:, :, 0:256], xk_v(0)[:, :, 0:256])
            nc.sync.dma_start(wv[:], wv_v)
            nc.sync.dma_start(xv_g[:, :, 0:256], xv_v(0)[:, :, 0:256])
            nc.sync.dma_start(xk_g[:, :, 256:512], xk_v(0)[:, :, 256:512])
            nc.sync.dma_start(xv_g[:, :, 256:512], xv_v(0)[:, :, 256:512])
            mark("dma0")

            proj_qk_half(xq_g, wq, QT[0], 0)
            proj_qk_half(xq_g, wq, QT[0], 1)
            proj_qk_half(xk_g, wk, KT[0], 0)
            attn_jseg(P00, range(0, 2))
            proj_v_jt(xv_g, 0)
            proj_v_jt(xv_g, 1)
            proj_qk_half(xk_g, wk, KT[0], 1)
            attn_jseg(P00, range(2, 4))
            proj_v_jt(xv_g, 2)
            proj_v_jt(xv_g, 3)
            mark("proj0")

            for g in range(1, G):
                xq_g, xk_g, xv_g = xg_tiles()
                nc.sync.dma_start(xq_g[:], xq_v(g))
                nc.sync.dma_start(xk_g[:], xk_v(g))
                nc.sync.dma_start(xv_g[:], xv_v(g))
                mark(f"dma{g}")
                # interleave attention with the projections so ACT keeps
                # streaming through the whole phase: P01/P02's segments (which
                # only need earlier groups' K) run during this group's
                # Q/K projections
                proj_qk(xq_g, wq, QT[g])
                attn_jseg(P01, range(4 * (g - 1), 4 * (g - 1) + 2))
                proj_qk(xk_g, wk, KT[g])
                attn_jseg(P01, range(4 * (g - 1) + 2, 4 * g))
                attn_jseg(P00, range(4 * g, 4 * g + 2))
                proj_v_jt(xv_g, 4 * g + 0)
                proj_v_jt(xv_g, 4 * g + 1)
                attn_jseg(P00, range(4 * g + 2, 4 * g + 4))
                proj_v_jt(xv_g, 4 * g + 2)
                proj_v_jt(xv_g, 4 * g + 3)
                mark(f"attn_pipe{g}")
            attn_jseg(P01, range(12, 16))
            mark("ramp_end")

            nc.sync.dma_start(wo[:], wo_v)

            for Pr in steady:
                attn_jseg(Pr, range(JT))
                mark(f"attn{Pr['hp']}{Pr['ic']}{'' if Pr['qh'] is None else 'ab'[Pr['qh']]}")
            pv_pump(budget=None)
            while FILLER:
                FILLER.pop(0)()
            mark("drain")

        if loop_cm is not None:
            with loop_cm:
                body()
        else:
            body()

    nc.compile()
    nc._phase_marks = marks
    return nc


def _in_maps(q, k, v, attention_mask, Wq, Wk, Wv, Wo, nloop=None):
    q = np.asarray(q, dtype=np.float32)
    k = np.asarray(k, dtype=np.float32)
    v = np.asarray(v, dtype=np.float32)
    Wq = np.asarray(Wq, dtype=np.float32)
    Wk = np.asarray(Wk, dtype=np.float32)
    Wv = np.asarray(Wv, dtype=np.float32)
    Wo = np.asarray(Wo, dtype=np.float32)
    mask = np.asarray(attention_mask)

    def tile_x(xb):
        # x[b] is [L, D]; device layout [g*P, DT*512] with
        # arr[g*P+p, dt*512+t] = x.T[dt*P+p, g*512+t]
        xt = xb.T.reshape(DT, P, G, 512).transpose(2, 1, 0, 3)
        return np.ascontiguousarray(xt.reshape(G * P, DT * 512)).astype(BF)

    def tile_w(wsl):
        # wsl is [D, M]; device layout [p, dt*M]
        wt = wsl.reshape(DT, P, HG * DH).transpose(1, 0, 2)
        return np.ascontiguousarray(wt.reshape(P, DT * HG * DH)).astype(BF)

    xT = {}
    for b in range(B):
        xT[("q", b)] = tile_x(q[b])
        xT[("k", b)] = tile_x(k[b])
        xT[("v", b)] = tile_x(v[b])

    in_maps = []
    for c in range(NC):
        b, hg = c // HG, c % HG
        rows = slice(hg * HG * DH, (hg + 1) * HG * DH)
        mbn = np.where(mask[b] == 0, np.float32(-1e9), np.float32(0.0))
        wo_sl = Wo[:, rows].T  # [HG*DH, D]
        wo_t = wo_sl.reshape(2, P, D).transpose(1, 0, 2).reshape(P, 2 * D)
        m = {
            "xqT": xT[("q", b)],
            "xkT": xT[("k", b)],
            "xvT": xT[("v", b)],
            "wqT": tile_w(Wq[rows].T),
            "wkT": tile_w(Wk[rows].T),
            "wvT": tile_w(Wv[rows].T),
            "woT": np.ascontiguousarray(wo_t).astype(BF),
            "mb": np.ascontiguousarray(mbn.reshape(JT, P).T),
        }
        if nloop is not None:
            m["nloop"] = np.array([[nloop]], dtype=np.int32)
        in_maps.append(m)
    return in_maps


def kernel(q, k, v, attention_mask, Wq, Wk, Wv, Wo):
    if "plain" not in _BUILT:
        _BUILT["plain"] = _build()
    nc = _BUILT["plain"]

    in_maps = _in_maps(q, k, v, attention_mask, Wq, Wk, Wv, Wo)
    res = run_bass_kernel_spmd(nc, in_maps, core_ids=list(range(NC)))
    kernel.last_results = res

    out = np.zeros((B, L, D), dtype=np.float64)
    for c in range(NC):
        part = res.results[c]["partial"].astype(np.float64)
        part = part.reshape(IC, P, 4, D).transpose(0, 2, 1, 3).reshape(L, D)
        out[c // HG] += part
    return out.astype(np.float32)


# revision 36
# speedup vs baseline: 1.3868x; 1.0552x over previous
"""Multi-head attention (B=2, L=2048, D=1024, H=16) on 8 TRN2 NeuronCores.

Sharding: batch x head-group. Core c handles batch c//4 and heads
4*(c%4) .. 4*(c%4)+3. Each core:
  - projects its q/k/v slices (transposed activations fed from host),
  - runs flash-style attention in the "S-transposed" layout
    (keys on partitions, queries on free dim) so no on-device transposes
    are ever needed,
  - computes a partial output projection against its Wo column slice.
Host sums the 4 partials per batch.

All DMA'd tensors (activations, weights, output partials) are bf16 to
halve HBM traffic; matmuls run bf16 (full PE rate at moving dim>=256)
with f32 PSUM accumulation. Softmax uses exp without max-subtraction
(scores are O(1) by construction); the attention mask folds into the
exp bias, and the softmax denominator comes for free from a ones-row
appended to V.

Emission is software-pipelined: group 0's projections run at 256-token
granularity ordered to match DMA arrival (wq, xq, wk, xk, wv, xv), so
the PE starts ~2.5us in; later projection groups are chunked by
512-token groups with the first two attention groups streaming behind
the input DMA.
"""
import sys

sys.path.insert(0, "/opt/trn_rl_repo")

import numpy as np
import ml_dtypes
from contextlib import ExitStack

import concourse.bass as bass
import concourse.mybir as mybir
import concourse.tile as tile
from concourse import bacc
from concourse.bass import ts
from concourse.bass_utils import run_bass_kernel_spmd

F32 = mybir.dt.float32
BF16 = mybir.dt.bfloat16
I32 = mybir.dt.int32
EXP = mybir.ActivationFunctionType.Exp
BF = ml_dtypes.bfloat16

B = 2
L = 2048
D = 1024
H = 16
DH = 64
HG = 4          # heads per core
NC = 8          # cores
P = 128
DT = D // P     # 8 d-tiles
JT = L // P     # 16 key tiles
IC = L // 512   # 4 query chunks of 512
G = 4           # projection token groups (512 tokens each)

_BUILT = {}


def _build(with_loop=False):
    nc = bacc.Bacc("TRN2", target_bir_lowering=False, debug=False, num_devices=1)

    # Host pre-tiles every input into the exact SBUF layout so each DMA row
    # is a large contiguous run (few fat descriptors instead of thousands of
    # 512B ones): x tensors as [group*P, DT*512], weights as [P, DT*M].
    xqT_d = nc.dram_tensor("xqT", (G * P, DT * 512), BF16, kind="ExternalInput").ap()
    xkT_d = nc.dram_tensor("xkT", (G * P, DT * 512), BF16, kind="ExternalInput").ap()
    xvT_d = nc.dram_tensor("xvT", (G * P, DT * 512), BF16, kind="ExternalInput").ap()
    wqT_d = nc.dram_tensor("wqT", (P, DT * HG * DH), BF16, kind="ExternalInput").ap()
    wkT_d = nc.dram_tensor("wkT", (P, DT * HG * DH), BF16, kind="ExternalInput").ap()
    wvT_d = nc.dram_tensor("wvT", (P, DT * HG * DH), BF16, kind="ExternalInput").ap()
    woT_d = nc.dram_tensor("woT", (P, 2 * D), BF16, kind="ExternalInput").ap()
    mb_d = nc.dram_tensor("mb", (P, JT), F32, kind="ExternalInput").ap()
    # partial output packed as [qchunk*P, 4tb*D] so each DMA row is 8KB
    # contiguous (128 fat descriptors per chunk instead of 512 thin ones)
    out_d = nc.dram_tensor("partial", (IC * P, 4 * D), BF16, kind="ExternalOutput").ap()
    if with_loop:
        nl_d = nc.dram_tensor("nloop", (1, 1), I32, kind="ExternalInput").ap()

    def xq_v(g):
        return xqT_d[ts(g, P), :].rearrange("p (dt t) -> p dt t", t=512)

    def xk_v(g):
        return xkT_d[ts(g, P), :].rearrange("p (dt t) -> p dt t", t=512)

    def xv_v(g):
        return xvT_d[ts(g, P), :].rearrange("p (dt t) -> p dt t", t=512)

    wq_v = wqT_d.rearrange("p (dt m) -> p dt m", m=HG * DH)
    wk_v = wkT_d.rearrange("p (dt m) -> p dt m", m=HG * DH)
    wv_v = wvT_d.rearrange("p (dt m) -> p dt m", m=HG * DH)
    wo_v = woT_d.rearrange("p (kt m) -> p kt m", m=D)

    marks = []

    def mark(label):
        marks.append((label, int(nc.get_next_instruction_name().split("-")[1])))

    with tile.TileContext(nc) as tc, ExitStack() as ctx:
        perm = ctx.enter_context(tc.tile_pool(name="perm", bufs=1))

        # resident weights
        wq = perm.tile([P, DT, HG * DH], BF16)
        wk = perm.tile([P, DT, HG * DH], BF16)
        wv = perm.tile([P, DT, HG * DH], BF16)
        wo = perm.tile([P, 2, D], BF16)
        mb = perm.tile([P, JT], F32)

        QT = [perm.tile([P, 2, 512], BF16, tag=f"QT{g}", name=f"QT{g}") for g in range(G)]
        KT = [perm.tile([P, 2, 512], BF16, tag=f"KT{g}", name=f"KT{g}") for g in range(G)]
        VT = [perm.tile([P, HG * (DH + 1)], BF16, tag=f"VT{j}", name=f"VT{j}") for j in range(JT)]
        OT = [perm.tile([P, 2, 512], BF16, tag=f"OT{g}", name=f"OT{g}") for g in range(G)]

        # ones columns of V (softmax denominator rows) — written once,
        # never clobbered by the V projection copies.
        for j in range(JT):
            vg = VT[j].rearrange("p (h c) -> p h c", c=DH + 1)
            nc.gpsimd.memset(vg[:, :, DH:DH + 1], 1.0)

        if with_loop:
            nl_sb = perm.tile([1, 1], I32)
            nc.sync.dma_start(nl_sb[:], nl_d[:])
            handles = []
            for eng in (nc.gpsimd, nc.scalar, nc.tensor, nc.vector, nc.sync):
                r = eng.alloc_register(f"nl_{eng.engine.name}")
                eng.reg_load(r, nl_sb[0:1, 0:1])
                handles.append(r)
            n_val = nc.snap(bass.RegisterHandles(handles), min_val=1, max_val=1 << 20)
            loop_cm = tc.For_i(0, n_val)
        else:
            loop_cm = None

        xpool = ctx.enter_context(tc.tile_pool(name="xg", bufs=4))
        spool = ctx.enter_context(tc.tile_pool(name="spool", bufs=2, space="PSUM"))
        spoolB = ctx.enter_context(tc.tile_pool(name="spoolB", bufs=1, space="PSUM"))
        stpv = ctx.enter_context(tc.tile_pool(name="stpv", bufs=1, space="PSUM"))
        opool = ctx.enter_context(tc.tile_pool(name="op", bufs=1, space="PSUM"))
        ptpool = ctx.enter_context(tc.tile_pool(name="pt", bufs=44))
        stpool = ctx.enter_context(tc.tile_pool(name="st", bufs=2))
        small = ctx.enter_context(tc.tile_pool(name="small", bufs=2))

        def body():
            # Scores-slot rotation: pool A (2 gens) during ramp; steady state
            # 3-slot scores rotation: spool's two gens + spoolB.
            rot = {"i": 0}

            def sslot():
                i = rot["i"] % 3
                rot["i"] += 1
                pool = spool if i < 2 else spoolB
                return pool.tile([P, 1024], F32, tag="s", name="s")

            def xg_tiles():
                xq_g = xpool.tile([P, DT, 512], BF16, tag="xg", name="xq_g")
                xk_g = xpool.tile([P, DT, 512], BF16, tag="xg", name="xk_g")
                xv_g = xpool.tile([P, DT, 512], BF16, tag="xg", name="xv_g")
                return xq_g, xk_g, xv_g

            def proj_qk_half(src, wt, dst, h):
                # 256-token half-group projection for Q/K (ramp only).
                ps = sslot()
                for p in range(2):
                    for d in range(DT):
                        nc.tensor.matmul(
                            ps[:, p * 512 + h * 256:p * 512 + h * 256 + 256],
                            wt[:, d, ts(p, P)], src[:, d, ts(h, 256)],
                            start=(d == 0), stop=(d == DT - 1),
                        )
                    nc.vector.tensor_copy(
                        dst[:, p, h * 256:h * 256 + 256],
                        ps[:, p * 512 + h * 256:p * 512 + h * 256 + 256],
                    )

            def proj_v_jt(xv_g, jt):
                psv = sslot()
                for d in range(DT):
                    nc.tensor.matmul(
                        psv[:, 0:HG * DH],
                        xv_g[:, d, ts(jt % 4, P)], wv[:, d, :],
                        start=(d == 0), stop=(d == DT - 1),
                    )
                vg = VT[jt].rearrange("p (h c) -> p h c", c=DH + 1)
                nc.vector.tensor_copy(
                    vg[:, :, 0:DH],
                    psv[:, 0:HG * DH].rearrange("p (h c) -> p h c", c=DH),
                )
                SCHED["vt_ready"] = jt

            def proj_qk(src, wt, dst):
                # out [pair-heads on partitions, 512 tokens]
                ps = sslot()
                for d in range(DT):
                    for p in range(2):
                        nc.tensor.matmul(
                            ps[:, ts(p, 512)], wt[:, d, ts(p, P)], src[:, d, :],
                            start=(d == 0), stop=(d == DT - 1),
                        )
                for p in range(2):
                    nc.vector.tensor_copy(dst[:, p, :], ps[:, ts(p, 512)])

            # ---- decoupled attention streams (pt ring) ----
            # The exp stream (scores -> exp -> pt ring) runs freely at ACT
            # rate; the PV stream consumes pts strictly in order at 2 matmuls
            # per exp: head A of a pair in the 2nd half of its own exp window,
            # head B in the 1st half of the next pair's window. This keeps a
            # single 1-bank pv accumulator (stpv) busy with no dead time.
            def mk_pair(hp, ic, qh, unlock):
                return {"hp": hp, "ic": ic, "qh": qh, "unlock": unlock,
                        "qn": 512 if qh is None else 256,
                        "pts": {}, "exp_done": -1, "pv": [None, None]}

            P00 = mk_pair(0, 0, None, [])
            P01 = mk_pair(0, 1, None, [])
            steady = [
                mk_pair(0, 2, None, []), mk_pair(0, 3, None, []),
                mk_pair(1, 0, None, [0, 1, 2, 3]),
                mk_pair(1, 1, None, [4, 5, 6, 7]),
                mk_pair(1, 2, None, [8, 9, 10, 11]),
                mk_pair(1, 3, 0, [12, 13]), mk_pair(1, 3, 1, [14, 15]),
            ]
            PVQ = []
            for Pr in [P00, P01] + steady:
                for h01 in range(2):
                    for j in range(JT):
                        PVQ.append((Pr, h01, j))
            FILLER = []

            ST4 = {}  # ic -> 4-block staging tile

            def oproj_half(tb, mc, sttile):
                ic = tb // 4
                if ic not in ST4:
                    ST4[ic] = stpool.tile([P, 4, D], BF16, tag="st", name="st")
                st = ST4[ic]
                pso = opool.tile([P, 512], F32, tag="pso", name="pso")
                for kt in range(2):
                    nc.tensor.matmul(
                        pso[:], OT[ic][:, kt, ts(tb % 4, P)],
                        wo[:, kt, ts(mc, 512)],
                        start=(kt == 0), stop=(kt == 1),
                    )
                nc.vector.tensor_copy(st[:, tb % 4, ts(mc, 512)], pso[:])
                if tb % 4 in (1, 3) and mc == 1:
                    half = (tb % 4) // 2
                    nc.sync.dma_start(
                        out_d[ts(ic, P), half * 2 * D:(half + 1) * 2 * D],
                        st.rearrange("p a b -> p (a b)")[:, half * 2 * D:(half + 1) * 2 * D])

            def norm(Pr, h01):
                hp, ic, qh, qn = Pr["hp"], Pr["ic"], Pr["qh"], Pr["qn"]
                q0 = 0 if qh is None else qh * 256
                # One fast copy frees the pv PSUM bank for the next block; the
                # 3-op normalization chain runs off-critical from the copy.
                pvc = small.tile([DH + 1, 512], F32, tag="pvc", name="pvc")
                nc.vector.tensor_copy(pvc[:, 0:qn], Pr["pv"][h01][:])
                rec = small.tile([1, 512], F32, tag="rec", name="rec")
                nc.vector.reciprocal(rec[:, 0:qn], pvc[DH:DH + 1, 0:qn])
                bc = small.tile([DH, 512], F32, tag="bc", name="bc")
                nc.gpsimd.partition_broadcast(bc[:, 0:qn], rec[:, 0:qn])
                nc.vector.tensor_mul(
                    out=OT[ic][ts(h01, DH), hp, q0:q0 + qn],
                    in0=pvc[0:DH, 0:qn], in1=bc[:, 0:qn],
                )
                if h01 == 1:
                    for tb in Pr["unlock"]:
                        sttile = {}
                        for mc in range(2):
                            FILLER.append(
                                lambda tb=tb, mc=mc, st=sttile: oproj_half(tb, mc, st))

            SCHED = {"vt_ready": -1}

            def pv_pump(cur_P=None, cur_j=None, budget=2):
                done = 0
                while PVQ and (budget is None or done < budget):
                    Pr, h01, j = PVQ[0]
                    if j > Pr["exp_done"] or j > SCHED["vt_ready"]:
                        break
                    if cur_P is Pr and j == cur_j:
                        break   # stay one exp behind the ACT stream
                    PVQ.pop(0)
                    if Pr["pv"][h01] is None:
                        Pr["pv"][h01] = stpv.tile(
                            [DH + 1, Pr["qn"]], F32, tag="pv", name="pv")
                    pt = Pr["pts"][j]
                    h = 2 * Pr["hp"] + h01
                    if Pr["qh"] is None:
                        rhs = pt[:, ts(h01, 512)]
                    else:
                        q0 = h01 * 512 + Pr["qh"] * 256
                        rhs = pt[:, q0:q0 + 256]
                    nc.tensor.matmul(
                        Pr["pv"][h01][:],
                        VT[j][:, h * (DH + 1):(h + 1) * (DH + 1)], rhs,
                        start=(j == 0), stop=(j == JT - 1),
                    )
                    if j == JT - 1:
                        norm(Pr, h01)
                    done += 1

            def attn_jseg(Pr, j_range, budget=2):
                hp, ic, qh = Pr["hp"], Pr["ic"], Pr["qh"]
                for j in j_range:
                    ps = sslot()
                    if qh is None:
                        nc.tensor.matmul(
                            ps[:, 0:512],
                            KT[j // 4][0:DH, hp, ts(j % 4, P)],
                            QT[ic][0:DH, hp, :],
                            start=True, stop=True,
                        )
                        nc.tensor.matmul(
                            ps[:, 512:1024],
                            KT[j // 4][DH:P, hp, ts(j % 4, P)],
                            QT[ic][DH:P, hp, :],
                            start=True, stop=True, tile_position=(DH, 0),
                        )
                        pin = ps[:]
                    else:
                        q0 = qh * 256
                        nc.tensor.matmul(
                            ps[:, q0:q0 + 256],
                            KT[j // 4][0:DH, hp, ts(j % 4, P)],
                            QT[ic][0:DH, hp, q0:q0 + 256],
                            start=True, stop=True,
                        )
                        nc.tensor.matmul(
                            ps[:, 512 + q0:512 + q0 + 256],
                            KT[j // 4][DH:P, hp, ts(j % 4, P)],
                            QT[ic][DH:P, hp, q0:q0 + 256],
                            start=True, stop=True, tile_position=(DH, 0),
                        )
                        pin = ps.rearrange("p (h q) -> p h q", h=2)[:, :, q0:q0 + 256]
                    pt = ptpool.tile([P, 1024], BF16, tag="pt", name="pt")
                    if qh is None:
                        nc.scalar.activation(
                            pt[:], pin, EXP, bias=mb[:, j:j + 1], scale=0.125,
                        )
                    else:
                        q0 = qh * 256
                        pto = pt.rearrange("p (h q) -> p h q", h=2)[:, :, q0:q0 + 256]
                        nc.scalar.activation(
                            pto, pin, EXP, bias=mb[:, j:j + 1], scale=0.125,
                        )
                    Pr["pts"][j] = pt
                    Pr["exp_done"] = j
                    pv_pump(Pr, j, budget)
                    if FILLER:
                        FILLER.pop(0)()

            # ---------- pipelined emission ----------
            # Ramp: DMA + projections stream behind the input DMA; group 0 at
            # 256-token granularity ordered to match DMA arrival; attention
            # pairs (0,0) and (0,1) trail the projections.
            xq_g, xk_g, xv_g = xg_tiles()
            nc.sync.dma_start(wq[:], wq_v)
            nc.sync.dma_start(xq_g[:, :, 0:256], xq_v(0)[:, :, 0:256])
            nc.sync.dma_start(wk[:], wk_v)
            nc.sync.dma_start(xk_g[:, :, 0:256], xk_v(0)[:, :, 0:256])
            nc.sync.dma_start(xq_g[:, :, 256:512], xq_v(0)[:, :, 256:512])
            nc.sync.dma_start(mb[:], mb_d[:])
            nc.sync.dma_start(xk_g[:, :, 256:512], xk_v(0)[:, :, 256:512])
            nc.sync.dma_start(wv[:], wv_v)
            nc.sync.dma_start(xv_g[:, :, 0:256], xv_v(0)[:, :, 0:256])
            nc.sync.dma_start(xv_g[:, :, 256:512], xv_v(0)[:, :, 256:512])
            mark("dma0")

            proj_qk_half(xq_g, wq, QT[0], 0)
            proj_qk_half(xq_g, wq, QT[0], 1)
            proj_qk_half(xk_g, wk, KT[0], 0)
            attn_jseg(P00, range(0, 2))
            proj_v_jt(xv_g, 0)
            proj_v_jt(xv_g, 1)
            proj_qk_half(xk_g, wk, KT[0], 1)
            attn_jseg(P00, range(2, 4))
            proj_v_jt(xv_g, 2)
            proj_v_jt(xv_g, 3)
            mark("proj0")

            for g in range(1, G):
                xq_g, xk_g, xv_g = xg_tiles()
                nc.sync.dma_start(xq_g[:], xq_v(g))
                nc.sync.dma_start(xk_g[:], xk_v(g))
                nc.sync.dma_start(xv_g[:], xv_v(g))
                mark(f"dma{g}")
                # interleave attention with the projections so ACT keeps
                # streaming through the whole phase: P01/P02's segments (which
                # only need earlier groups' K) run during this group's
                # Q/K projections
                proj_qk(xq_g, wq, QT[g])
                attn_jseg(P01, range(4 * (g - 1), 4 * (g - 1) + 2))
                proj_qk(xk_g, wk, KT[g])
                attn_jseg(P01, range(4 * (g - 1) + 2, 4 * g))
                attn_jseg(P00, range(4 * g, 4 * g + 2))
                proj_v_jt(xv_g, 4 * g + 0)
                proj_v_jt(xv_g, 4 * g + 1)
                attn_jseg(P00, range(4 * g + 2, 4 * g + 4))
                proj_v_jt(xv_g, 4 * g + 2)
                proj_v_jt(xv_g, 4 * g + 3)
                mark(f"attn_pipe{g}")
            attn_jseg(P01, range(12, 16))
            mark("ramp_end")

            nc.sync.dma_start(wo[:], wo_v)

            for Pr in steady:
                attn_jseg(Pr, range(JT))
                mark(f"attn{Pr['hp']}{Pr['ic']}{'' if Pr['qh'] is None else 'ab'[Pr['qh']]}")
            pv_pump(budget=None)
            while FILLER:
                FILLER.pop(0)()
            mark("drain")

        if loop_cm is not None:
            with loop_cm:
                body()
        else:
            body()

    nc.compile()
    nc._phase_marks = marks
    return nc


def _in_maps(q, k, v, attention_mask, Wq, Wk, Wv, Wo, nloop=None):
    q = np.asarray(q, dtype=np.float32)
    k = np.asarray(k, dtype=np.float32)
    v = np.asarray(v, dtype=np.float32)
    Wq = np.asarray(Wq, dtype=np.float32)
    Wk = np.asarray(Wk, dtype=np.float32)
    Wv = np.asarray(Wv, dtype=np.float32)
    Wo = np.asarray(Wo, dtype=np.float32)
    mask = np.asarray(attention_mask)

    def tile_x(xb):
        # x[b] is [L, D]; device layout [g*P, DT*512] with
        # arr[g*P+p, dt*512+t] = x.T[dt*P+p, g*512+t]
        xt = xb.T.reshape(DT, P, G, 512).transpose(2, 1, 0, 3)
        return np.ascontiguousarray(xt.reshape(G * P, DT * 512)).astype(BF)

    def tile_w(wsl):
        # wsl is [D, M]; device layout [p, dt*M]
        wt = wsl.reshape(DT, P, HG * DH).transpose(1, 0, 2)
        return np.ascontiguousarray(wt.reshape(P, DT * HG * DH)).astype(BF)

    xT = {}
    for b in range(B):
        xT[("q", b)] = tile_x(q[b])
        xT[("k", b)] = tile_x(k[b])
        xT[("v", b)] = tile_x(v[b])

    in_maps = []
    for c in range(NC):
        b, hg = c // HG, c % HG
        rows = slice(hg * HG * DH, (hg + 1) * HG * DH)
        mbn = np.where(mask[b] == 0, np.float32(-1e9), np.float32(0.0))
        wo_sl = Wo[:, rows].T  # [HG*DH, D]
        wo_t = wo_sl.reshape(2, P, D).transpose(1, 0, 2).reshape(P, 2 * D)
        m = {
            "xqT": xT[("q", b)],
            "xkT": xT[("k", b)],
            "xvT": xT[("v", b)],
            "wqT": tile_w(Wq[rows].T),
            "wkT": tile_w(Wk[rows].T),
            "wvT": tile_w(Wv[rows].T),
            "woT": np.ascontiguousarray(wo_t).astype(BF),
            "mb": np.ascontiguousarray(mbn.reshape(JT, P).T),
        }
        if nloop is not None:
            m["nloop"] = np.array([[nloop]], dtype=np.int32)
        in_maps.append(m)
    return in_maps


def kernel(q, k, v, attention_mask, Wq, Wk, Wv, Wo):
    if "plain" not in _BUILT:
        _BUILT["plain"] = _build()
    nc = _BUILT["plain"]

    in_maps = _in_maps(q, k, v, attention_mask, Wq, Wk, Wv, Wo)
    res = run_bass_kernel_spmd(nc, in_maps, core_ids=list(range(NC)))
    kernel.last_results = res

    out = np.zeros((B, L, D), dtype=np.float64)
    for c in range(NC):
        part = res.results[c]["partial"].astype(np.float64)
        part = part.reshape(IC, P, 4, D).transpose(0, 2, 1, 3).reshape(L, D)
        out[c // HG] += part
    return out.astype(np.float32)
